# revision 1
# baseline (speedup 1.0000x reference)
"""LocalFrameAttentionWithDiffuser on 8 TRN2 NeuronCores.

Sharding: head-parallel. Each core computes 2 of the 16 heads end-to-end
(QKV projection for its 128 hd-dims, chunked local attention, partial
output projection Y_c = O_c @ Wo[c-slice]); the host sums the 8 partial
Y tensors (bias bo is fed only to core 0 so the sum adds it once).

Shapes (hardcoded from the problem):
  x [1,16,256,1024] -> tokens T=4096, D=1024, H=16 heads, HD=64,
  chunks C=4 of L=1024 tokens; chunk i attends to chunks {i-1, i}
  (chunk 0 only to itself).

Device layout notes:
  - everything flows transposed: X^T [D, T] is a host-prepared input so
    projections produce Q^T/K^T [hd, T] directly (hd on partitions).
  - S^T = K^T.T @ Q^T per (chunk, head) with ctx on partitions, so the
    softmax sum over ctx is computed by appending a ones-column to V in
    the AV matmul (row 64 of the AV PSUM accumulates sum(exp(s))).
  - chunk 0's missing previous chunk is handled by simply not issuing
    those ctx tiles (exactly reproduces the -inf mask).
  - matmuls use float32r (full-rate fp32 path on the PE).
"""

import os
from contextlib import ExitStack

import numpy as np

import concourse.bass as bass
import concourse.tile as tile
from concourse import bacc, mybir
from concourse.bass_utils import run_bass_kernel_spmd

F32 = mybir.dt.float32
F32R = mybir.dt.float32r

B, F, N, D = 1, 16, 256, 1024
H, HD = 16, 64
CS = 4
C = F // CS            # 4 chunks
L = CS * N             # 1024 tokens per chunk
T = F * N              # 4096 tokens
NCORES = 8
HPC = H // NCORES      # 2 heads per core
HDB = HPC * HD         # 128 hd dims per core
SCALE = 1.0 / np.sqrt(HD)

TOK_TILE = 512         # moving-dim tile (fp32 max)
NDT = D // 128         # 8 contraction tiles for projections
NJT = T // TOK_TILE    # 8 token tiles
NCT = T // 128         # 32 ctx tiles of 128


def _r(ap):
    return ap.bitcast(F32R)


def build_kernel(nc, tc, outs, ins, ctx, phases=3):
    xt, wq, wk, wv, wo, bo, ident = (
        ins["xt"], ins["wq"], ins["wk"], ins["wv"], ins["wo"], ins["bo"],
        ins["ident"],
    )
    y = outs["y"]

    # persistent pools: bufs=1, every tile gets a distinct name (= its own slot)
    wpool = ctx.enter_context(tc.tile_pool(name="weights", bufs=1))
    qk_pool = ctx.enter_context(tc.tile_pool(name="qk", bufs=1))
    v_pool = ctx.enter_context(tc.tile_pool(name="v", bufs=1))
    ot_pool = ctx.enter_context(tc.tile_pool(name="ot", bufs=1))
    ybias_pool = ctx.enter_context(tc.tile_pool(name="ybias", bufs=1))
    # cycling pools: shared tag -> bufs slots
    xpool = ctx.enter_context(tc.tile_pool(name="xt", bufs=16))
    vstage_pool = ctx.enter_context(tc.tile_pool(name="vstage", bufs=2))
    a_pool = ctx.enter_context(tc.tile_pool(name="attn", bufs=12))
    sum_pool = ctx.enter_context(tc.tile_pool(name="sums", bufs=8))
    bc_pool = ctx.enter_context(tc.tile_pool(name="bcast", bufs=4))
    yout_pool = ctx.enter_context(tc.tile_pool(name="yout", bufs=6))
    ps_pool = ctx.enter_context(tc.tile_pool(name="ps", bufs=8, space="PSUM"))
    proj_ps = vtr_ps = s_ps = o_ps = y_ps = ps_pool

    # ---- persistent weights / constants (loaded at first use) ----
    wq_sb = [wpool.tile([128, HDB], F32R, name=f"wq{d}") for d in range(NDT)]
    wk_sb = [wpool.tile([128, HDB], F32R, name=f"wk{d}") for d in range(NDT)]
    wv_sb = [wpool.tile([128, HDB], F32R, name=f"wv{d}") for d in range(NDT)]
    wo_sb = wpool.tile([128, D], F32R, tag="wo")
    id_sb = wpool.tile([128, 128], F32, tag="id")
    ones_col = wpool.tile([128, 1], F32, tag="ones")
    nc.vector.memset(ones_col[:], 1.0)
    bo_bc = ybias_pool.tile([128, D], F32)

    # persistent activations
    qt_sb = qk_pool.tile([128, T], F32R, tag="qt")   # Q^T (2 heads stacked)
    kt_sb = qk_pool.tile([128, T], F32R, tag="kt")   # K^T
    ot_sb = ot_pool.tile([128, T], F32R)             # O^T normalized
    # V per ctx tile: [128 tok, 65] (64 hd + ones column), per head
    v_sb = [[v_pool.tile([128, HD + 1], F32R, name=f"v{h}_{ct}") for ct in range(NCT)]
            for h in range(HPC)]

    # ---- phase 1: projections (per 512-token tile) ----
    for j in range(NJT):
        tok = bass.ts(j, TOK_TILE)
        xt_t = [xpool.tile([128, TOK_TILE], F32R, tag="x", name=f"xt{j}_{d}") for d in range(NDT)]
        for d in range(NDT):
            nc.sync.dma_start(xt_t[d][:], xt[d * 128:(d + 1) * 128, tok].bitcast(F32R))
            if j == 0:
                nc.sync.dma_start(wq_sb[d][:], wq[d * 128:(d + 1) * 128, :].bitcast(F32R))
                nc.sync.dma_start(wk_sb[d][:], wk[d * 128:(d + 1) * 128, :].bitcast(F32R))
                nc.sync.dma_start(wv_sb[d][:], wv[d * 128:(d + 1) * 128, :].bitcast(F32R))
        if j == 0:
            nc.sync.dma_start(id_sb[:], ident[:, :])
        if j == 2:
            nc.sync.dma_start(wo_sb[:], wo[:, :].bitcast(F32R))
            nc.sync.dma_start(bo_bc[:], bo[0:1, :].broadcast_to([128, D]))
        q_ps = proj_ps.tile([128, TOK_TILE], F32, tag="ps", name=f"qps{j}")
        k_ps = proj_ps.tile([128, TOK_TILE], F32, tag="ps", name=f"kps{j}")
        vt_ps = proj_ps.tile([128, TOK_TILE], F32, tag="ps", name=f"vps{j}")
        for d in range(NDT):
            st, sp = d == 0, d == NDT - 1
            nc.tensor.matmul(q_ps[:], wq_sb[d][:], xt_t[d][:], start=st, stop=sp)
            nc.tensor.matmul(k_ps[:], wk_sb[d][:], xt_t[d][:], start=st, stop=sp)
            nc.tensor.matmul(vt_ps[:], wv_sb[d][:], xt_t[d][:], start=st, stop=sp)
        nc.vector.tensor_copy(qt_sb[:, tok], q_ps[:])
        nc.vector.tensor_copy(kt_sb[:, tok], k_ps[:])
        vt_stage = vstage_pool.tile([128, TOK_TILE], F32, tag="vs", name=f"vst{j}")
        nc.vector.tensor_copy(vt_stage[:], vt_ps[:])
        # transpose V^T -> V in 128x128 blocks; split the two heads
        for kblk in range(TOK_TILE // 128):
            ct = j * (TOK_TILE // 128) + kblk
            vtr = vtr_ps.tile([128, 128], F32, tag="ps", name=f"vtr{j}_{kblk}")
            nc.tensor.transpose(vtr[:], vt_stage[:, bass.ts(kblk, 128)], id_sb[:])
            for h in range(HPC):
                nc.vector.tensor_copy(v_sb[h][ct][:, 0:HD], vtr[:, h * HD:(h + 1) * HD])
                nc.gpsimd.tensor_copy(v_sb[h][ct][:, HD:HD + 1], ones_col[:])

    # ---- phase 2+3: attention per chunk, then its slice of the output proj ----
    if phases < 2:
        return
    for c in range(C):
        cts = list(range(max(0, 8 * (c - 1)), 8 * (c + 1)))  # ctx tiles (128 tok)
        for th in range(L // TOK_TILE):  # 2 query halves per chunk
            tok0 = c * L + th * TOK_TILE
            tok = bass.ds(tok0, TOK_TILE)
            for h in range(HPC):
                hr = slice(h * HD, (h + 1) * HD)
                o_acc = o_ps.tile([HD + 1, TOK_TILE], F32, tag="ps", name=f"ops{c}_{th}_{h}")
                for ci, ct in enumerate(cts):
                    s_t = s_ps.tile([128, TOK_TILE], F32, tag="ps", name=f"sps{c}_{th}_{h}_{ci}")
                    nc.tensor.matmul(
                        s_t[:], kt_sb[hr, bass.ts(ct, 128)], qt_sb[hr, tok],
                        start=True, stop=True,
                    )
                    a_t = a_pool.tile([128, TOK_TILE], F32R, tag="a", name=f"a{c}_{th}_{h}_{ci}")
                    nc.scalar.activation(
                        a_t[:], s_t[:], mybir.ActivationFunctionType.Exp, scale=SCALE
                    )
                    nc.tensor.matmul(
                        o_acc[:], v_sb[h][ct][:], a_t[:],
                        start=(ci == 0), stop=(ci == len(cts) - 1),
                    )
                # normalize: rows 0:64 / row 64
                s_sum = sum_pool.tile([1, TOK_TILE], F32, tag="s", name=f"ssum{c}_{th}_{h}")
                nc.vector.reciprocal(s_sum[:], o_acc[HD:HD + 1, :])
                r_bc = bc_pool.tile([HD, TOK_TILE], F32, tag="bc", name=f"bc{c}_{th}_{h}")
                nc.gpsimd.partition_broadcast(r_bc[:], s_sum[0:1, :])
                nc.vector.tensor_mul(ot_sb[hr, tok], o_acc[0:HD, :], r_bc[:])
            # output projection for this half-chunk's 4 token tiles
            for m in ([] if phases < 3 else range(8 * c + 4 * th, 8 * c + 4 * (th + 1))):
                for dh in range(D // TOK_TILE):
                    yp = y_ps.tile([128, TOK_TILE], F32, tag="ps", name=f"yps{m}_{dh}")
                    nc.tensor.matmul(
                        yp[:], ot_sb[:, bass.ts(m, 128)],
                        wo_sb[:, bass.ts(dh, TOK_TILE)],
                        start=True, stop=True,
                    )
                    y_sb = yout_pool.tile([128, TOK_TILE], F32, tag="yo", name=f"yo{m}_{dh}")
                    nc.vector.tensor_add(y_sb[:], yp[:], bo_bc[:, bass.ts(dh, TOK_TILE)])
                    nc.sync.dma_start(y[bass.ts(m, 128), bass.ts(dh, TOK_TILE)], y_sb[:])


_CACHE = {}


def _build(phases=3):
    if ("nc", phases) in _CACHE:
        return _CACHE[("nc", phases)]
    nc = bacc.Bacc(
        "TRN2",
        target_bir_lowering=False,
        debug=False,
        enable_asserts=False,
        num_devices=NCORES,
    )
    ins = {
        "xt": nc.dram_tensor("xt", [D, T], F32, kind="ExternalInput").ap(),
        "wq": nc.dram_tensor("wq", [D, HDB], F32, kind="ExternalInput").ap(),
        "wk": nc.dram_tensor("wk", [D, HDB], F32, kind="ExternalInput").ap(),
        "wv": nc.dram_tensor("wv", [D, HDB], F32, kind="ExternalInput").ap(),
        "wo": nc.dram_tensor("wo", [HDB, D], F32, kind="ExternalInput").ap(),
        "bo": nc.dram_tensor("bo", [1, D], F32, kind="ExternalInput").ap(),
        "ident": nc.dram_tensor("ident", [128, 128], F32, kind="ExternalInput").ap(),
    }
    outs = {"y": nc.dram_tensor("y", [T, D], F32, kind="ExternalOutput").ap()}
    with tile.TileContext(nc, trace_sim=False) as tc:
        with ExitStack() as kctx:
            build_kernel(nc, tc, outs, ins, kctx, phases=phases)
    nc.compile()
    _CACHE[("nc", phases)] = nc
    return nc


def make_in_maps(x, Wq, Wk, Wv, Wo, bo):
    xt = np.ascontiguousarray(
        np.asarray(x, dtype=np.float32).reshape(T, D).T
    )
    ident = np.eye(128, dtype=np.float32)
    bo = np.asarray(bo, dtype=np.float32).reshape(1, D)
    zeros_bo = np.zeros_like(bo)
    in_maps = []
    for core in range(NCORES):
        hs = slice(core * HDB, (core + 1) * HDB)
        in_maps.append({
            "xt": xt,
            "wq": np.ascontiguousarray(np.asarray(Wq, np.float32)[:, hs]),
            "wk": np.ascontiguousarray(np.asarray(Wk, np.float32)[:, hs]),
            "wv": np.ascontiguousarray(np.asarray(Wv, np.float32)[:, hs]),
            "wo": np.ascontiguousarray(np.asarray(Wo, np.float32)[hs, :]),
            "bo": bo if core == 0 else zeros_bo,
            "ident": ident,
        })
    return in_maps


def kernel(x, Wq, Wk, Wv, Wo, bo, _trace=False, _tmpdir=None):
    nc = _build()
    in_maps = make_in_maps(x, Wq, Wk, Wv, Wo, bo)
    res = run_bass_kernel_spmd(
        nc, in_maps, core_ids=list(range(NCORES)),
        trace=_trace, tmpdir=_tmpdir,
        **({"trace_cores": list(range(NCORES))} if _trace else {}),
    )
    if _trace:
        kernel.last_results = res
    y = np.zeros((T, D), dtype=np.float32)
    for r in res.results:
        y += r["y"]
    return y.reshape(B, F, N, D)



# revision 13
# speedup vs baseline: 1.4508x; 1.4508x over previous
"""LocalFrameAttentionWithDiffuser on 8 TRN2 NeuronCores.

Sharding: head-parallel. Each core computes 2 of the 16 heads end-to-end
(QKV projection for its 128 hd-dims, chunked local attention, partial
output projection Y_c = O_c @ Wo[c-slice]); the host sums the 8 partial
Y tensors and adds the bias.

Shapes (hardcoded from the problem):
  x [1,16,256,1024] -> tokens T=4096, D=1024, H=16 heads, HD=64,
  chunks C=4 of L=1024 tokens; chunk i attends to chunks {i-1, i}
  (chunk 0 only to itself).

Device pipeline (per core):
  - Q/K projections run as fp8e4 DoubleRow matmuls (contraction 2x128 per
    instruction, 0.5 cycles/row): X and Wq/Wk are host-quantized to fp8,
    W scaled by 64 to stay in normal fp8 range. Scores come out scaled by
    64*64, compensated in the exp's scale argument. Score-path quantization
    noise perturbs softmax weights randomly and averages out over the
    2048-token context; the V path (whose noise does NOT average out)
    stays bf16.
  - V projection is bf16, directly in [tok, hd] layout (X^T tiles as
    stationary, Wv as moving), with a constant ones column appended so
    the AV matmul also produces softmax denominators.
  - Scores S^T = K^T.T @ Q^T per (chunk, head) with ctx on partitions;
    two 128-ctx tiles share one 2-bank PSUM tile so a single Exp
    activation covers 1024 elements (halves Act-engine overhead).
  - AV uses A-tiles as stationary and V [128, 65] as moving, producing
    O as [tok, hd] plus the denominator; normalization is a reciprocal +
    per-partition tensor_scalar_mul, then a bf16 PE transpose back to
    [hd, tok] for the output projection.
  - Emission is software-pipelined: AV/normalize/out-proj of the previous
    (chunk, tok-half) and the next chunk's projections are interleaved
    between score/exp pairs so the Activation engine (the critical
    resource at ~116us busy) rarely starves.
"""

from contextlib import ExitStack

import numpy as np
import ml_dtypes

import concourse.bass as bass
import concourse.tile as tile
from concourse import bacc, mybir
from concourse.bass_utils import run_bass_kernel_spmd

F32 = mybir.dt.float32
BF16 = mybir.dt.bfloat16
FP8 = mybir.dt.float8e4

B, F, N, D = 1, 16, 256, 1024
H, HD = 16, 64
CS = 4
C = F // CS            # 4 chunks
L = CS * N             # 1024 tokens per chunk
T = F * N              # 4096 tokens
NCORES = 8
HPC = H // NCORES      # 2 heads per core
HDB = HPC * HD         # 128 hd dims per core
QK_FP8 = False         # fp8 DoubleRow Q/K projections (faster, noisier)
WSCALE = 64.0          # fp8 weight pre-scale (Wq/Wk only)
EXP_SCALE = (1.0 / np.sqrt(HD)) / (WSCALE * WSCALE if QK_FP8 else 1.0)

NDT = D // 128         # 8 contraction tiles (bf16 path)
NDR = D // 256         # 4 DoubleRow contraction tiles (fp8 path)
NTB = T // 128         # 32 128-token blocks
NPAIR = NTB // 2       # 16 ctx-tile pairs


def build_kernel(nc, tc, outs, ins, ctx):
    y = outs["y"]

    sb = ctx.enter_context(tc.tile_pool(name="sb", bufs=1))
    a_pool = ctx.enter_context(tc.tile_pool(name="attn", bufs=26))
    small = ctx.enter_context(tc.tile_pool(name="small", bufs=8))
    otp = ctx.enter_context(tc.tile_pool(name="otp", bufs=6))
    ysb_pool = ctx.enter_context(tc.tile_pool(name="ysb", bufs=4))
    ps = ctx.enter_context(tc.tile_pool(name="ps", bufs=1, space="PSUM"))

    # ---- persistent SBUF tensors ----
    xt = [sb.tile([128, T], BF16, name=f"xt{d}") for d in range(NDT)]
    if QK_FP8:
        xdr = [sb.tile([128, 2, T], FP8, name=f"xdr{r}") for r in range(NDR)]
        wq_dr = [sb.tile([128, 2, HDB], FP8, name=f"wqdr{r}") for r in range(NDR)]
        wk_dr = [sb.tile([128, 2, HDB], FP8, name=f"wkdr{r}") for r in range(NDR)]
    else:
        wq_bf = sb.tile([128, NDT * HDB], BF16)
        wk_bf = sb.tile([128, NDT * HDB], BF16)
    wv_sb = sb.tile([128, NDT * HDB], BF16)
    wo_sb = sb.tile([128, D], BF16)
    id_sb = sb.tile([128, 128], BF16)
    qt_sb = sb.tile([128, T], BF16)     # Q^T x 64 (2 heads stacked)
    kt_sb = sb.tile([128, T], BF16)     # K^T x 64
    # V per (head, ctx-tile-pair): [128 tok, 2*(64 hd + ones col)]
    v_sb = [[sb.tile([128, 2 * (HD + 1)], BF16, name=f"v{h}_{p}")
             for p in range(NPAIR)] for h in range(HPC)]

    # ---- input DMAs (SP queue), ordered for pipeline startup ----
    # 1) Q/K inputs + weights (first exp depends only on these)
    if QK_FP8:
        for r in range(NDR):
            nc.sync.dma_start(xdr[r][:, :, 0:2048], ins[f"xdr{r}"][:, :, 0:2048])
            nc.sync.dma_start(wq_dr[r][:], ins[f"wq{r}"][:])
            nc.sync.dma_start(wk_dr[r][:], ins[f"wk{r}"][:])
    else:
        nc.sync.dma_start(wq_bf[:], ins["wqb"][:])
        nc.sync.dma_start(wk_bf[:], ins["wkb"][:])
    nc.sync.dma_start(id_sb[:], ins["ident"][:])
    # 2) first quarter of X bf16 + Wv (chunk 0's projections)
    for d in range(NDT):
        nc.sync.dma_start(xt[d][:, 0:1024], ins["xt"][d * 128:(d + 1) * 128, 0:1024])
    nc.sync.dma_start(wv_sb[:], ins["wv"][:])
    # 3) the rest
    for d in range(NDT):
        nc.sync.dma_start(xt[d][:, 1024:2048], ins["xt"][d * 128:(d + 1) * 128, 1024:2048])
    nc.sync.dma_start(wo_sb[:], ins["wo"][:])
    if QK_FP8:
        for r in range(NDR):
            nc.sync.dma_start(xdr[r][:, :, 2048:T], ins[f"xdr{r}"][:, :, 2048:T])
    for d in range(NDT):
        nc.sync.dma_start(xt[d][:, 2048:T], ins["xt"][d * 128:(d + 1) * 128, 2048:T])

    # ones columns of every V tile (cols 0:64/65:129 overwritten later)
    for h in range(HPC):
        for p in range(NPAIR):
            nc.gpsimd.memset(v_sb[h][p][:], 1.0)

    # ---- emission helpers ----
    def proj_qk(j, w, dst, nm):
        """Projection of one 512-token tile for Q or K (fp8 DoubleRow or
        bf16 depending on QK_FP8; `w` is the per-mode weight handle)."""
        def emit():
            tok = bass.ts(j, 512)
            pp = ps.tile([128, 512], F32, tag="bk", bufs=4, name=f"p{nm}{j}")
            if QK_FP8:
                for r in range(NDR):
                    nc.tensor.matmul(pp[:], w[r][:], xdr[r][:, :, tok],
                                     start=(r == 0), stop=(r == NDR - 1),
                                     perf_mode=mybir.MatmulPerfMode.DoubleRow)
            else:
                for d in range(NDT):
                    nc.tensor.matmul(pp[:], w[:, bass.ts(d, HDB)],
                                     xt[d][:, tok],
                                     start=(d == 0), stop=(d == NDT - 1))
            nc.vector.tensor_copy(dst[:, tok], pp[:])
        return emit

    def proj_v(tb):
        """bf16 V projection of one 128-token block -> v_sb pair halves."""
        def emit():
            tok = bass.ts(tb, 128)
            pv = ps.tile([128, HDB], F32, tag="bk", bufs=4, name=f"pv{tb}")
            for d in range(NDT):
                nc.tensor.matmul(pv[:], xt[d][:, tok],
                                 wv_sb[:, bass.ts(d, HDB)],
                                 start=(d == 0), stop=(d == NDT - 1))
            for h in range(HPC):
                nc.vector.tensor_copy(
                    v_sb[h][tb // 2][:, (tb % 2) * (HD + 1):(tb % 2) * (HD + 1) + HD],
                    pv[:, h * HD:(h + 1) * HD])
        return emit

    a_tiles = {}

    def av_block(c, th, tb, cts):
        """AV + normalize for both heads of one 128-token block, then
        transpose to [hd, tok], out-projection and the output DMA."""
        def emit():
            tokblk = c * 8 + th * 4 + tb
            o2n = {}
            for h in range(HPC):
                o2 = ps.tile([128, HD + 1], F32, tag="bk", bufs=4,
                             name=f"o2_{tokblk}_{h}")
                for ci, ct in enumerate(cts):
                    half = ct % 2
                    nc.tensor.matmul(
                        o2[:],
                        a_tiles[(c, th, h, ct // 2)][:, half * 512 + tb * 128:
                                                     half * 512 + tb * 128 + 128],
                        v_sb[h][ct // 2][:, half * (HD + 1):half * (HD + 1) + HD + 1],
                        start=(ci == 0), stop=(ci == len(cts) - 1))
                rec = small.tile([128, 1], F32, tag="rec", name=f"rc{tokblk}_{h}")
                nc.vector.reciprocal(rec[:], o2[:, HD:HD + 1])
                on = otp.tile([128, HD], BF16, tag="on", name=f"on{tokblk}_{h}")
                nc.vector.tensor_scalar_mul(on[:], o2[:, 0:HD], rec[:])
                o2n[h] = on
            ot_ps = ps.tile([128, 128], BF16, tag="bk", bufs=4, name=f"otp{tokblk}")
            for h in range(HPC):
                nc.tensor.transpose(ot_ps[h * HD:(h + 1) * HD, :], o2n[h][:],
                                    id_sb[:], tile_position=(0, h * HD))
            ot = otp.tile([128, 128], BF16, tag="ot", name=f"ot{tokblk}")
            nc.vector.tensor_copy(ot[:], ot_ps[:])
            ysb = ysb_pool.tile([128, D], BF16, tag="y", name=f"ysb{tokblk}")
            for dh in range(2):
                yp = ps.tile([128, 512], F32, tag="bk", bufs=4,
                             name=f"yp{tokblk}_{dh}")
                nc.tensor.matmul(yp[:], ot[:], wo_sb[:, bass.ts(dh, 512)],
                                 start=True, stop=True)
                nc.vector.tensor_copy(ysb[:, bass.ts(dh, 512)], yp[:])
            nc.sync.dma_start(y[bass.ts(tokblk, 128), :], ysb[:])
        return emit

    # ---- software-pipelined main loop ----
    filler = []

    def drain(k):
        nonlocal filler
        for f in filler[:k]:
            f()
        filler = filler[k:]

    wqh = wq_dr if QK_FP8 else wq_bf
    wkh = wk_dr if QK_FP8 else wk_bf

    # prologue: chunk 0 Q/K projections inline; V via filler
    for j in (0, 1):
        proj_qk(j, wkh, kt_sb, "k")()
        proj_qk(j, wqh, qt_sb, "q")()
    filler.extend(proj_v(tb) for tb in range(8))

    blocks = [(c, th, h) for c in range(C) for th in range(2) for h in range(HPC)]
    for bi, (c, th, h) in enumerate(blocks):
        cts = list(range(max(0, 8 * (c - 1)), 8 * (c + 1)))
        pairs = sorted({ct // 2 for ct in cts})
        tok = bass.ds(c * L + th * 512, 512)

        # enqueue deferred work: next chunk's projections, split across the
        # first two blocks of this chunk so X DMAs have time to land
        if th == 0 and h == 0 and c + 1 < C:
            for j in (2 * (c + 1), 2 * (c + 1) + 1):
                filler.append(proj_qk(j, wkh, kt_sb, "k"))
                filler.append(proj_qk(j, wqh, qt_sb, "q"))
            filler.extend(proj_v(tb) for tb in range(8 * (c + 1), 8 * (c + 1) + 4))
        if th == 0 and h == 1 and c + 1 < C:
            filler.extend(proj_v(tb) for tb in range(8 * (c + 1) + 4, 8 * (c + 2)))

        per = max(1, -(-len(filler) // len(pairs)))
        for p in pairs:
            sc = ps.tile([128, 1024], F32, tag="sc", bufs=2, name=f"sc{bi}_{p}")
            for half in range(2):
                ct = 2 * p + half
                nc.tensor.matmul(
                    sc[:, bass.ts(half, 512)],
                    kt_sb[h * HD:(h + 1) * HD, bass.ts(ct, 128)],
                    qt_sb[h * HD:(h + 1) * HD, tok],
                    start=True, stop=True)
            at = a_pool.tile([128, 1024], BF16, tag="a", name=f"a{bi}_{p}")
            nc.scalar.activation(at[:], sc[:],
                                 mybir.ActivationFunctionType.Exp,
                                 scale=EXP_SCALE)
            a_tiles[(c, th, h, p)] = at
            drain(per)

        if h == HPC - 1:
            filler.extend(av_block(c, th, tb, cts) for tb in range(4))

    drain(len(filler))


_CACHE = {}


def _build():
    if "nc" in _CACHE:
        return _CACHE["nc"]
    nc = bacc.Bacc(
        "TRN2",
        target_bir_lowering=False,
        debug=False,
        enable_asserts=False,
        num_devices=NCORES,
    )
    ins = {
        "xt": nc.dram_tensor("xt", [D, T], BF16, kind="ExternalInput").ap(),
        "wv": nc.dram_tensor("wv", [128, NDT * HDB], BF16, kind="ExternalInput").ap(),
        "wo": nc.dram_tensor("wo", [HDB, D], BF16, kind="ExternalInput").ap(),
        "ident": nc.dram_tensor("ident", [128, 128], BF16, kind="ExternalInput").ap(),
    }
    if QK_FP8:
        for r in range(NDR):
            ins[f"xdr{r}"] = nc.dram_tensor(f"xdr{r}", [128, 2, T], FP8,
                                            kind="ExternalInput").ap()
            ins[f"wq{r}"] = nc.dram_tensor(f"wq{r}", [128, 2, HDB], FP8,
                                           kind="ExternalInput").ap()
            ins[f"wk{r}"] = nc.dram_tensor(f"wk{r}", [128, 2, HDB], FP8,
                                           kind="ExternalInput").ap()
    else:
        ins["wqb"] = nc.dram_tensor("wqb", [128, NDT * HDB], BF16,
                                    kind="ExternalInput").ap()
        ins["wkb"] = nc.dram_tensor("wkb", [128, NDT * HDB], BF16,
                                    kind="ExternalInput").ap()
    outs = {"y": nc.dram_tensor("y", [T, D], BF16, kind="ExternalOutput").ap()}
    with tile.TileContext(nc, trace_sim=False) as tc:
        with ExitStack() as kctx:
            build_kernel(nc, tc, outs, ins, kctx)
    nc.compile()
    _CACHE["nc"] = nc
    return nc


def make_in_maps(x, Wq, Wk, Wv, Wo, bo):
    xt32 = np.asarray(x, dtype=np.float32).reshape(T, D).T  # [D, T]
    xt = np.ascontiguousarray(xt32).astype(ml_dtypes.bfloat16)
    ident = np.eye(128, dtype=np.float32).astype(ml_dtypes.bfloat16)
    if QK_FP8:
        # DoubleRow layout: xdr[r][p, i, t] = X^T[r*256 + i*128 + p, t]
        xdr8 = xt32.reshape(NDR, 2, 128, T).transpose(0, 2, 1, 3)
        xdrs = [np.ascontiguousarray(xdr8[r]).astype(ml_dtypes.float8_e4m3)
                for r in range(NDR)]

    def wdr(w, hs):
        m = np.asarray(w, np.float32)[:, hs] * WSCALE    # [D, 128]
        m = m.reshape(NDR, 2, 128, HDB).transpose(0, 2, 1, 3)
        return [np.ascontiguousarray(m[r]).astype(ml_dtypes.float8_e4m3)
                for r in range(NDR)]

    def wrow(w, hs):
        """[D, 128] weight slice -> [128, NDT*128] d-tiled bf16 layout."""
        m = np.asarray(w, np.float32)[:, hs]
        return np.ascontiguousarray(
            m.reshape(NDT, 128, HDB).transpose(1, 0, 2).reshape(128, NDT * HDB)
        ).astype(ml_dtypes.bfloat16)

    in_maps = []
    for core in range(NCORES):
        hs = slice(core * HDB, (core + 1) * HDB)
        wo = np.ascontiguousarray(
            np.asarray(Wo, np.float32)[hs, :]).astype(ml_dtypes.bfloat16)
        im = {"xt": xt, "wv": wrow(Wv, hs), "wo": wo, "ident": ident}
        if QK_FP8:
            wqs, wks = wdr(Wq, hs), wdr(Wk, hs)
            for r in range(NDR):
                im[f"xdr{r}"] = xdrs[r]
                im[f"wq{r}"] = wqs[r]
                im[f"wk{r}"] = wks[r]
        else:
            im["wqb"] = wrow(Wq, hs)
            im["wkb"] = wrow(Wk, hs)
        in_maps.append(im)
    return in_maps


def kernel(x, Wq, Wk, Wv, Wo, bo, _trace=False, _tmpdir=None):
    nc = _build()
    in_maps = make_in_maps(x, Wq, Wk, Wv, Wo, bo)
    res = run_bass_kernel_spmd(
        nc, in_maps, core_ids=list(range(NCORES)),
        trace=_trace, tmpdir=_tmpdir,
        **({"trace_cores": list(range(NCORES))} if _trace else {}),
    )
    if _trace:
        kernel.last_results = res
    y = np.zeros((T, D), dtype=np.float32)
    for r in res.results:
        y += np.asarray(r["y"], dtype=np.float32)
    y += np.asarray(bo, dtype=np.float32).reshape(1, D)
    return y.reshape(B, F, N, D)


# revision 21
# speedup vs baseline: 1.7578x; 1.2116x over previous
"""LocalFrameAttentionWithDiffuser on 8 TRN2 NeuronCores.

Sharding: head-parallel. Each core computes 2 of the 16 heads end-to-end
(QKV projection for its 128 hd-dims, chunked local attention, partial
output projection Y_c = O_c @ Wo[c-slice]); the host sums the 8 partial
Y tensors and adds the bias.

Shapes (hardcoded from the problem):
  x [1,16,256,1024] -> tokens T=4096, D=1024, H=16 heads, HD=64,
  chunks C=4 of L=1024 tokens; chunk i attends to chunks {i-1, i}
  (chunk 0 only to itself).

Device pipeline (per core):
  - All three projections run as fp8e4 DoubleRow matmuls (contraction
    2x128 per instruction, 0.5 cycles/row) with residual compensation:
    X ~= X8 + X8lo and W ~= W8 + W8lo (all host-quantized fp8, same
    scale family), and the PSUM accumulates X8@W8 + X8@W8lo + X8lo@W8.
    This is 0.75x the PE cost of bf16 at bf16-class accuracy (~0.2%);
    plain fp8 is NOT usable anywhere in the data path because
    multiplicative noise on scores/A/V lands ~1:1 in the output (the
    output is a random-sign sum, so noise does not average out).
  - Wq/Wk are pre-scaled by 64 and Wv by 32 so the fp8 weights sit in
    the normal range; the Q/K scale is compensated in the exp's scale
    argument, and the V scale cancels in the softmax normalize (the
    denominator ones-column is also 32).
  - V is produced directly in [tok, hd] layout (X8 tiles stationary,
    Wv8 moving) - no transpose needed.
  - Scores S^T = K^T.T @ Q^T per (chunk, head) with ctx on partitions;
    two 128-ctx tiles share one 2-bank PSUM tile so a single Exp
    activation covers 1024 elements (halves Act-engine overhead).
  - AV uses A-tiles as stationary (bf16) and V [128, 65] as moving
    (64 hd dims + the 32s column -> softmax denominators for free),
    producing O as [tok, hd]; normalization is a reciprocal +
    per-partition tensor_scalar_mul, then one bf16 PE transpose per
    128-token block back to [hd, tok] for the output projection.
  - Emission is software-pipelined: AV/normalize/out-proj of the
    previous (chunk, tok-half) and the next chunk's projections are
    interleaved between score/exp pairs so the Activation engine
    (the critical resource at ~116us busy) rarely starves.
"""

from contextlib import ExitStack

import numpy as np
import ml_dtypes

import concourse.bass as bass
import concourse.tile as tile
from concourse import bacc, mybir
from concourse.bass_utils import run_bass_kernel_spmd

F32 = mybir.dt.float32
BF16 = mybir.dt.bfloat16
FP8 = mybir.dt.float8e4

B, F, N, D = 1, 16, 256, 1024
H, HD = 16, 64
CS = 4
C = F // CS            # 4 chunks
L = CS * N             # 1024 tokens per chunk
T = F * N              # 4096 tokens
NCORES = 8
HPC = H // NCORES      # 2 heads per core
HDB = HPC * HD         # 128 hd dims per core
QKSCALE = 64.0         # fp8 pre-scale for Wq/Wk
VSCALE = 32.0          # fp8 pre-scale for Wv (cancels in normalize)
EXP_SCALE = (1.0 / np.sqrt(HD)) / (QKSCALE * QKSCALE)

NDR = D // 256         # 4 DoubleRow contraction tiles
NTB = T // 128         # 32 128-token blocks
NPAIR = NTB // 2       # 16 ctx-tile pairs
NW = 6 * NDR           # weight blocks: (q,k,v) x (hi,lo) x NDR


def build_kernel(nc, tc, outs, ins, ctx):
    y = outs["y"]

    sb = ctx.enter_context(tc.tile_pool(name="sb", bufs=1))
    a_pool = ctx.enter_context(tc.tile_pool(name="attn", bufs=26))
    small = ctx.enter_context(tc.tile_pool(name="small", bufs=8))
    otp = ctx.enter_context(tc.tile_pool(name="otp", bufs=6))
    ysb_pool = ctx.enter_context(tc.tile_pool(name="ysb", bufs=4))
    ps = ctx.enter_context(tc.tile_pool(name="ps", bufs=1, space="PSUM"))

    # ---- persistent SBUF tensors ----
    x8 = [sb.tile([128, 2, T], FP8, name=f"x8_{r}") for r in range(NDR)]
    x8lo = [sb.tile([128, 2, T], FP8, name=f"x8lo_{r}") for r in range(NDR)]
    # packed weights: [(q,k,v) x (hi,lo)] x NDR blocks of [128, 2, 128]
    wpk = sb.tile([128, NW, 2, HDB], FP8)

    def wblk(proj, lo, r):
        return wpk[:, (proj * 2 + lo) * NDR + r, :, :]

    wo_sb = sb.tile([128, D], BF16)
    id_sb = sb.tile([128, 128], BF16)
    qt_sb = sb.tile([128, T], BF16)     # Q^T x 64 (2 heads stacked)
    kt_sb = sb.tile([128, T], BF16)     # K^T x 64
    # V x 32 per (head, ctx-tile-pair): [128 tok, 2*(64 hd + 32s col)] bf16
    v_sb = [[sb.tile([128, 2 * (HD + 1)], BF16, name=f"v{h}_{p}")
             for p in range(NPAIR)] for h in range(HPC)]

    # warm up the Act engine's exp table before real data arrives
    warm = small.tile([128, 8], F32, tag="warm")
    nc.vector.memset(warm[:], 0.0)
    nc.scalar.activation(warm[:], warm[:], mybir.ActivationFunctionType.Exp,
                         scale=1.0)

    # ---- input DMAs (SP queue), ordered for pipeline startup ----
    nc.sync.dma_start(wpk[:], ins["wpk"][:])
    for r in range(NDR):
        nc.sync.dma_start(x8[r][:, :, 0:1024], ins[f"x8_{r}"][:, :, 0:1024])
    for r in range(NDR):
        nc.sync.dma_start(x8lo[r][:, :, 0:1024], ins[f"x8lo_{r}"][:, :, 0:1024])
    nc.sync.dma_start(id_sb[:], ins["ident"][:])
    for r in range(NDR):
        nc.sync.dma_start(x8[r][:, :, 1024:2048], ins[f"x8_{r}"][:, :, 1024:2048])
        nc.sync.dma_start(x8lo[r][:, :, 1024:2048],
                          ins[f"x8lo_{r}"][:, :, 1024:2048])
    nc.sync.dma_start(wo_sb[:], ins["wo"][:])
    for r in range(NDR):
        nc.sync.dma_start(x8[r][:, :, 2048:T], ins[f"x8_{r}"][:, :, 2048:T])
        nc.sync.dma_start(x8lo[r][:, :, 2048:T], ins[f"x8lo_{r}"][:, :, 2048:T])

    # 32s columns of every V tile (cols 0:64/65:129 overwritten later)
    for h in range(HPC):
        for p in range(NPAIR):
            nc.gpsimd.memset(v_sb[h][p][:], VSCALE)

    DR = mybir.MatmulPerfMode.DoubleRow

    def proj_qk(j, proj, dst, nm):
        """3-term fp8 DoubleRow projection of one 512-token tile (Q or K)."""
        def emit():
            tok = bass.ts(j, 512)
            pp = ps.tile([128, 512], F32, tag="bk", bufs=4, name=f"p{nm}{j}")
            mms = ([(wblk(proj, 0, r), x8[r]) for r in range(NDR)]
                   + [(wblk(proj, 1, r), x8[r]) for r in range(NDR)]
                   + [(wblk(proj, 0, r), x8lo[r]) for r in range(NDR)])
            for i, (w, xx) in enumerate(mms):
                nc.tensor.matmul(pp[:], w, xx[:, :, tok],
                                 start=(i == 0), stop=(i == len(mms) - 1),
                                 perf_mode=DR)
            nc.vector.tensor_copy(dst[:, tok], pp[:])
        return emit

    def proj_v(tb):
        """3-term fp8 DoubleRow V projection of one 128-token block,
        directly in [tok, hd] layout -> v_sb pair halves (bf16)."""
        def emit():
            tok = bass.ts(tb, 128)
            pv = ps.tile([128, HDB], F32, tag="bk", bufs=4, name=f"pv{tb}")
            mms = ([(x8[r], wblk(2, 0, r)) for r in range(NDR)]
                   + [(x8[r], wblk(2, 1, r)) for r in range(NDR)]
                   + [(x8lo[r], wblk(2, 0, r)) for r in range(NDR)])
            for i, (xx, w) in enumerate(mms):
                nc.tensor.matmul(pv[:], xx[:, :, tok], w,
                                 start=(i == 0), stop=(i == len(mms) - 1),
                                 perf_mode=DR)
            for h in range(HPC):
                nc.vector.tensor_copy(
                    v_sb[h][tb // 2][:, (tb % 2) * (HD + 1):(tb % 2) * (HD + 1) + HD],
                    pv[:, h * HD:(h + 1) * HD])
        return emit

    a_tiles = {}
    on_tiles = {}

    def av_head(c, th, h, tb, cts):
        """AV + normalize for one head of one 128-token block; both heads
        write the same [128, 128] normalized-O tile (cols h*64..)."""
        def emit():
            tokblk = c * 8 + th * 4 + tb
            o2 = ps.tile([128, HD + 1], F32, tag="bk", bufs=4,
                         name=f"o2_{tokblk}_{h}")
            for ci, ct in enumerate(cts):
                half = ct % 2
                nc.tensor.matmul(
                    o2[:],
                    a_tiles[(c, th, h, ct // 2)][:, half * 512 + tb * 128:
                                                 half * 512 + tb * 128 + 128],
                    v_sb[h][ct // 2][:, half * (HD + 1):half * (HD + 1) + HD + 1],
                    start=(ci == 0), stop=(ci == len(cts) - 1))
            rec = small.tile([128, 1], F32, tag="rec", name=f"rc{tokblk}_{h}")
            nc.vector.reciprocal(rec[:], o2[:, HD:HD + 1])
            if h == 0:
                on_tiles[tokblk] = otp.tile([128, 2 * HD], BF16, tag="on",
                                            bufs=10, name=f"on{tokblk}")
            nc.vector.tensor_scalar_mul(on_tiles[tokblk][:, h * HD:(h + 1) * HD],
                                        o2[:, 0:HD], rec[:])
        return emit

    def finish_block(c, th, tb):
        """Transpose normalized O back to [hd, tok], out-project, DMA out."""
        def emit():
            tokblk = c * 8 + th * 4 + tb
            ot_ps = ps.tile([128, 128], BF16, tag="bk", bufs=4, name=f"otp{tokblk}")
            nc.tensor.transpose(ot_ps[:], on_tiles[tokblk][:], id_sb[:])
            ot = otp.tile([128, 128], BF16, tag="ot", name=f"ot{tokblk}")
            nc.vector.tensor_copy(ot[:], ot_ps[:])
            ysb = ysb_pool.tile([128, D], BF16, tag="y", name=f"ysb{tokblk}")
            for dh in range(2):
                yp = ps.tile([128, 512], F32, tag="bk", bufs=4,
                             name=f"yp{tokblk}_{dh}")
                nc.tensor.matmul(yp[:], ot[:], wo_sb[:, bass.ts(dh, 512)],
                                 start=True, stop=True)
                nc.vector.tensor_copy(ysb[:, bass.ts(dh, 512)], yp[:])
            nc.sync.dma_start(y[bass.ts(tokblk, 128), :], ysb[:])
        return emit

    # ---- software-pipelined main loop ----
    filler = []

    def drain(k):
        nonlocal filler
        for f in filler[:k]:
            f()
        filler = filler[k:]

    # prologue: chunk 0 Q/K projections inline; V via filler
    for j in (0, 1):
        proj_qk(j, 1, kt_sb, "k")()
        proj_qk(j, 0, qt_sb, "q")()
    filler.extend(proj_v(tb) for tb in range(8))

    blocks = [(c, th, h) for c in range(C) for th in range(2) for h in range(HPC)]
    for bi, (c, th, h) in enumerate(blocks):
        cts = list(range(max(0, 8 * (c - 1)), 8 * (c + 1)))
        pairs = sorted({ct // 2 for ct in cts})
        tok = bass.ds(c * L + th * 512, 512)

        # enqueue deferred work: next chunk's projections, split across the
        # first two blocks of this chunk so X DMAs have time to land
        if th == 0 and h == 0 and c + 1 < C:
            for j in (2 * (c + 1), 2 * (c + 1) + 1):
                filler.append(proj_qk(j, 1, kt_sb, "k"))
                filler.append(proj_qk(j, 0, qt_sb, "q"))
            filler.extend(proj_v(tb) for tb in range(8 * (c + 1), 8 * (c + 1) + 4))
        if th == 0 and h == 1 and c + 1 < C:
            filler.extend(proj_v(tb) for tb in range(8 * (c + 1) + 4, 8 * (c + 2)))

        # drain rate: 1/pair in steady state, more when the end nears
        pairs_left = sum(
            len(range(max(0, 8 * (cc - 1)), 8 * (cc + 1))) // 2
            for (cc, _, _) in blocks[bi:])
        per = max(1, -(-(len(filler) + 8) // max(1, pairs_left)))
        for p in pairs:
            sc = ps.tile([128, 1024], F32, tag="sc", bufs=2, name=f"sc{bi}_{p}")
            for half in range(2):
                ct = 2 * p + half
                nc.tensor.matmul(
                    sc[:, bass.ts(half, 512)],
                    kt_sb[h * HD:(h + 1) * HD, bass.ts(ct, 128)],
                    qt_sb[h * HD:(h + 1) * HD, tok],
                    start=True, stop=True)
            at = a_pool.tile([128, 1024], BF16, tag="a", name=f"a{bi}_{p}")
            nc.scalar.activation(at[:], sc[:],
                                 mybir.ActivationFunctionType.Exp,
                                 scale=EXP_SCALE)
            a_tiles[(c, th, h, p)] = at
            drain(per)

        filler.extend(av_head(c, th, h, tb, cts) for tb in range(4))
        if h == HPC - 1:
            filler.extend(finish_block(c, th, tb) for tb in range(4))

    drain(len(filler))


_CACHE = {}


def _build():
    if "nc" in _CACHE:
        return _CACHE["nc"]
    nc = bacc.Bacc(
        "TRN2",
        target_bir_lowering=False,
        debug=False,
        enable_asserts=False,
        num_devices=NCORES,
    )
    ins = {
        "wpk": nc.dram_tensor("wpk", [128, NW, 2, HDB], FP8,
                              kind="ExternalInput").ap(),
        "wo": nc.dram_tensor("wo", [HDB, D], BF16, kind="ExternalInput").ap(),
        "ident": nc.dram_tensor("ident", [128, 128], BF16,
                                kind="ExternalInput").ap(),
    }
    for r in range(NDR):
        ins[f"x8_{r}"] = nc.dram_tensor(f"x8_{r}", [128, 2, T], FP8,
                                        kind="ExternalInput").ap()
        ins[f"x8lo_{r}"] = nc.dram_tensor(f"x8lo_{r}", [128, 2, T], FP8,
                                          kind="ExternalInput").ap()
    outs = {"y": nc.dram_tensor("y", [T, D], BF16, kind="ExternalOutput").ap()}
    with tile.TileContext(nc, trace_sim=False) as tc:
        with ExitStack() as kctx:
            build_kernel(nc, tc, outs, ins, kctx)
    nc.compile()
    _CACHE["nc"] = nc
    return nc


def make_in_maps(x, Wq, Wk, Wv, Wo, bo):
    f8 = ml_dtypes.float8_e4m3
    xt32 = np.asarray(x, dtype=np.float32).reshape(T, D).T  # [D, T]
    # DoubleRow layout [r][p, i, t] = X^T[r*256 + i*128 + p, t], hi + residual
    xdr = np.ascontiguousarray(xt32.reshape(NDR, 2, 128, T).transpose(0, 2, 1, 3))
    x8 = xdr.astype(f8)
    x8lo = (xdr - x8.astype(np.float32)).astype(f8)
    ident = np.eye(128, dtype=np.float32).astype(ml_dtypes.bfloat16)

    def wdr(w, hs, scale):
        """[D, 128] weight slice -> DR layout [NDR, 128, 2, 128] hi + lo."""
        m = np.asarray(w, np.float32)[:, hs] * scale
        m = np.ascontiguousarray(m.reshape(NDR, 2, 128, HDB).transpose(0, 2, 1, 3))
        hi = m.astype(f8)
        lo = (m - hi.astype(np.float32)).astype(f8)
        return hi, lo

    in_maps = []
    for core in range(NCORES):
        hs = slice(core * HDB, (core + 1) * HDB)
        wo = np.ascontiguousarray(
            np.asarray(Wo, np.float32)[hs, :]).astype(ml_dtypes.bfloat16)
        # packed weight tensor: [(q,k,v) x (hi,lo)] x NDR of [128, 2, 128]
        blocks = []
        for w, scale in ((Wq, QKSCALE), (Wk, QKSCALE), (Wv, VSCALE)):
            hi, lo = wdr(w, hs, scale)
            blocks.append(hi)
            blocks.append(lo)
        wpk = np.ascontiguousarray(
            np.stack(blocks, axis=0)            # [6, NDR, 128, 2, 128]
            .transpose(2, 0, 1, 3, 4)           # [128, 6, NDR, 2, 128]
            .reshape(128, NW, 2, HDB))
        im = {"wpk": wpk, "wo": wo, "ident": ident}
        for r in range(NDR):
            im[f"x8_{r}"] = x8[r]
            im[f"x8lo_{r}"] = x8lo[r]
        in_maps.append(im)
    return in_maps


def kernel(x, Wq, Wk, Wv, Wo, bo, _trace=False, _tmpdir=None):
    nc = _build()
    in_maps = make_in_maps(x, Wq, Wk, Wv, Wo, bo)
    res = run_bass_kernel_spmd(
        nc, in_maps, core_ids=list(range(NCORES)),
        trace=_trace, tmpdir=_tmpdir,
        **({"trace_cores": list(range(NCORES))} if _trace else {}),
    )
    if _trace:
        kernel.last_results = res
    y = np.zeros((T, D), dtype=np.float32)
    for r in res.results:
        y += np.asarray(r["y"], dtype=np.float32)
    y += np.asarray(bo, dtype=np.float32).reshape(1, D)
    return y.reshape(B, F, N, D)
